# revision 61
# baseline (speedup 1.0000x reference)
"""Trainium2 Bass kernel for CurvSelfAttention.

Reference computation (per batch b):
    Q = hs @ Wq + bq ; K = hs @ Wk + bk ; V = hs @ Wv + bv      # [S, H]
    s = sigmoid(hs @ Ws + bs) * 0.2 + 0.9                        # [S, NH*G]
    Q[:, h*64+g*8+r] *= s[:, h*8+g]
    per head h: ctx_h = softmax(Q_h K_h^T / 8) V_h               # [S, 64]
    out = concat_h(ctx_h)                                        # [S, NH*64]

Sharding over 8 cores: core c = (b = c // 2, hh = c % 2); each core owns
batch b and heads hh*8 .. hh*8+8 (512 output columns). No collectives.

HW facts this design leans on (measured):
  - matmul cost ~ N_moving/2.4ns + ~4ns, independent of stationary size;
    a stalled PE drops to a slow p-state (~2x) until ~3us of continuous work
  - LDW mostly hides behind the previous matmul
  - ACT (the only exp engine) runs 1 elem/cycle/lane: (N+~350)/1.2 ns,
    giving a hard ~270us/core floor for the 33.5M softmax exps
  - start=True clears the whole PSUM bank (has_written bits), so only the
    FIRST accumulation chain touching a bank may send it
  - per-DMA queue overhead ~0.6-1.1us: few big multi-segment DMAs win;
    the prefix is HBM-bound (~7.4MB over 3 queues)

Per-core algorithm:
  hsT [P,KT,S]    <- hs arrives HOST-TRANSPOSED [H,S]; 4 t-quarter DMAs
  sxc             <- Ws^T-stationary matmuls -> tanh((x+bs)/2) (sigmoid via
                     tanh: exp/tanh share ONE ACT table set); x8 expand +
                     affine s = 0.1*tanh + 1.0 via SEL matmul (host-packed)
  K^T [j, t]      <- W^T-stationary matmuls (pair rows: h0 d 0-63, h1 64-127)
  qpad[hp]        <- [P, NSSB, 2, W2]: per window [q0 | q1], epilogue writes
                     (ps+bq)*sexp into the live half, other half zeroed, so
                     the scores stationary is the full [128,128] K-pair chunk
  V [t, j]        <- hsT-chunk-stationary matmuls + ones col (denominator)
  scores psum     [128t, 1024]: head0 cols 0-511, head1 512-1023 (2 banks)
  probs           ONE Exp N=1024 per t-chunk covering both heads -> bf16 ring
  ctx^T           probs chunk as STATIONARY, V [128,65] moving (N=65):
                   psum [128s, 4cc, 65] per head accumulates over t-chunks;
                   col 64 = denom; normalize+assemble straight from psum
                   (no PE transposes, no evacuation), DMA per s-chunk
  warmup          garbage matmuls keep the PE p-state hot through DMA stalls

Loop: hp x ssb windows of 512 s, 16 t-chunks each; ALL projection work
(V, K/Q of later quarters/head-pairs, scale/SEL) drains as deadline-ordered
filler pieces inside the windows (emission order IS the dependency order:
each piece must drain >= one window before its first consumer is emitted).
"""

import os
import sys

sys.path.insert(0, "/opt/trn_rl_repo")

import numpy as np
import ml_dtypes
from collections import deque
from contextlib import ExitStack

import concourse.bass as bass
import concourse.bacc as bacc
import concourse.tile as tile
from concourse import mybir
from concourse import bass_utils

F32 = mybir.dt.float32
BF16 = mybir.dt.bfloat16
AF = mybir.ActivationFunctionType
ALU = mybir.AluOpType

P = 128          # SBUF partitions
NB = 512         # matmul moving free-dim block
W2 = 512         # attention s-window
HD = 64          # head dim
G = 8            # groups per head
RING = 6         # probs ring chunks
LAG = 5          # ctx trails exp by this many chunks
SC_MIN, SC_MAX = 0.9, 1.1


def build_bass(S=2048, H=1024, NHL=8):
    """Build the per-core Bass module. NHL = local heads; JL = NHL*64."""
    JL = NHL * HD
    GL = NHL * G           # compact scale channels (64)
    KT = H // P            # contraction k-tiles (8)
    JB = JL // P           # j row-blocks == head pairs (4)
    NTB = S // NB          # 512-wide t blocks (4)
    NTC = S // P           # 128-wide t chunks (16)
    NSSB = S // W2         # s windows per head pair (4)
    HP = NHL // 2          # head pairs (4)

    nc = bacc.Bacc(trn_type="TRN2", target_bir_lowering=False, debug=False,
                   num_devices=8)

    hs = nc.dram_tensor("hs", [H, S], BF16, kind="ExternalInput").ap()
    wq = nc.dram_tensor("wq", [H, JL], BF16, kind="ExternalInput").ap()
    wk = nc.dram_tensor("wk", [H, JL], BF16, kind="ExternalInput").ap()
    wv = nc.dram_tensor("wv", [H, JL], BF16, kind="ExternalInput").ap()
    # ws host-packed to [P, KT, GL] so it loads in ONE contiguous DMA
    ws = nc.dram_tensor("ws", [P, KT * GL], BF16, kind="ExternalInput").ap()
    # cb: host-packed per-partition consts [P, 9] f32:
    #   col 0 rows 0..GL-1 = bs/2; cols 1..4 = bk[jb]; cols 5..8 = bq[jb]
    cb = nc.dram_tensor("cb", [P, 1 + 2 * JB], F32,
                        kind="ExternalInput").ap()
    bs = nc.dram_tensor("bs", [GL], F32, kind="ExternalInput").ap()
    bv = nc.dram_tensor("bv", [JL], F32, kind="ExternalInput").ap()
    sel = nc.dram_tensor("sel", [GL + 1, JL], BF16, kind="ExternalInput").ap()
    out = nc.dram_tensor("out", [S, JL], F32, kind="ExternalOutput").ap()

    with tile.TileContext(nc) as tc, ExitStack() as ctx:
        cpool = ctx.enter_context(tc.tile_pool(name="consts", bufs=1))
        qkpool = ctx.enter_context(tc.tile_pool(name="qk", bufs=1))
        vpool = ctx.enter_context(tc.tile_pool(name="v", bufs=1))
        sxpool = ctx.enter_context(tc.tile_pool(name="sexp", bufs=1))
        hpool = ctx.enter_context(tc.tile_pool(name="hsT", bufs=1))
        wpool = ctx.enter_context(tc.tile_pool(name="wts", bufs=1))
        ppsum = ctx.enter_context(tc.tile_pool(name="ppsum", bufs=2,
                                               space="PSUM"))
        ptmp = ctx.enter_context(tc.tile_pool(name="ptmp", bufs=2))

        # persistent activation tensors
        # qpad[hp]: [P, NSSB, 2, W2]; window ssb holds [q0 | q1] side by side
        # so ONE N=1024 scores matmul covers both heads. q0 slot: rows 0-63
        # live (head 2hp), rows 64-127 zero; q1 slot: rows 64-127 live.
        qpad = [qkpool.tile([P, NSSB, 2, W2], BF16, tag=f"qp{hp}",
                            name=f"qp{hp}") for hp in range(HP)]
        k_sb = [qkpool.tile([P, S], BF16, tag=f"k{jb}", name=f"k{jb}")
                for jb in range(JB)]
        # V as [t-chunk][128, NHL, 65]; col 64 of each head = ones (denom)
        v_sb = [vpool.tile([P, NHL, HD + 1], BF16, tag=f"v{i}", name=f"v{i}")
                for i in range(NTC)]
        sexp = [sxpool.tile([P, S], BF16, tag=f"sx{jb}", name=f"sx{jb}")
                for jb in range(JB)]
        sxc = ptmp.tile([GL + 1, S], BF16, tag="sxc", bufs=1)

        # ---- input loads. hs arrives pre-transposed from the host ([H, S])
        # and streams as t-QUARTERS (all 8 k-chunks of quarter q land by
        # ~4.9*(q+1) us) so the scale/K/Q chains can chase arrivals. Three
        # DMA queues run in parallel:
        #   sync:   hsT quarter 0, wv+bvb, hsT quarters 1-3
        #   scalar: small consts only (keeps ACT free for tanh early)
        #   gpsimd: wk, wq (1MB each)
        # DVE (idle early) does the one-time memsets.
        # hsT lives in ONE tile [P, KT, S]; each t-quarter loads as a single
        # 8-segment DMA (1KB segments) — per-DMA queue overhead is ~1.1us,
        # so few big DMAs beat many small ones
        hsTall = hpool.tile([P, KT, S], BF16, tag="hsTall", name="hsTall")
        hsT = [hsTall[:, k, :] for k in range(KT)]
        hs_r = hs.rearrange("(k p) s -> p k s", p=P)
        for tb in range(NTB):
            nc.sync.dma_start(hsTall[:, :, tb * NB:(tb + 1) * NB],
                              hs_r[:, :, tb * NB:(tb + 1) * NB])

        # scalar queue: 3 contiguous const DMAs (done in ~2us, then ACT
        # loads its exp/tanh table once — the ONLY table set used: sigmoid
        # is computed as 0.5*tanh(x/2)+0.5 with the affine folded into SEL)
        wsall = wpool.tile([P, KT, GL], BF16, tag="wsall")
        nc.scalar.dma_start(
            wsall[:], ws.rearrange("p (k c) -> p k c", k=KT))
        ws_sb = [wsall[:, k, :] for k in range(KT)]
        cb_sb = cpool.tile([P, 1 + 2 * JB], F32, tag="cb")
        nc.scalar.dma_start(cb_sb[:], cb)
        # bs gets its own contiguous [GL,1] tile: the ACT bias operand must
        # not be a strided column slice
        bs_sb = cpool.tile([GL, 1], F32, tag="bs")
        nc.scalar.dma_start(bs_sb[:], bs.rearrange("(a b) -> a b", b=1))
        bs_sb = bs_sb[:]
        bk_sb = [cb_sb[:, 1 + jb:2 + jb] for jb in range(JB)]
        bq_sb = [cb_sb[:, 1 + JB + jb:2 + JB + jb] for jb in range(JB)]
        sel_sb = cpool.tile([GL + 1, JL], BF16, tag="sel")
        nc.scalar.dma_start(sel_sb[:], sel)
        # V weights: one big DMA on the scalar queue (done before tanh needs
        # the engine)
        wvall = wpool.tile([P, KT, JL], BF16, tag="wvall", name="wvall")
        nc.scalar.dma_start(wvall[:], wv.rearrange("(k p) c -> p k c", p=P))
        wv_sb = [wvall[:, k, :] for k in range(KT)]

        # gpsimd queue: Q weights first (prefix-critical), then K, then bvb
        wqall = wpool.tile([P, KT, JL], BF16, tag="wqall", name="wqall")
        nc.gpsimd.dma_start(wqall[:], wq.rearrange("(k p) c -> p k c", p=P))
        wq_sb = [wqall[:, k, :] for k in range(KT)]
        wkall = wpool.tile([P, KT, JL], BF16, tag="wkall", name="wkall")
        nc.gpsimd.dma_start(wkall[:], wk.rearrange("(k p) c -> p k c", p=P))
        wk_sb = [wkall[:, k, :] for k in range(KT)]
        bvb = cpool.tile([P, JL], F32, tag="bvb")
        nc.gpsimd.dma_start(
            bvb[:], bv.rearrange("(a b) -> a b", a=1).broadcast_to([P, JL]))

        # one-time memsets: only window-(0,*)-critical ones go on DVE (its
        # FIFO otherwise delays the prefix epilogues); the rest go on gpsimd
        # which is idle once its DMA descriptors are queued
        nc.vector.memset(sxc[GL:GL + 1, :], 1.0)
        nc.vector.memset(qpad[0][HD:P, :, 0, :], 0.0)
        nc.vector.memset(qpad[0][0:HD, :, 1, :], 0.0)
        for i in range(NTC):
            nc.vector.memset(v_sb[i][:, :, HD], 1.0)
        for hp in range(1, HP):
            nc.gpsimd.memset(qpad[hp][HD:P, :, 0, :], 0.0)
            nc.gpsimd.memset(qpad[hp][0:HD, :, 1, :], 0.0)

        # ---- projection work units (4 matmuls each), emitted either in
        # the prefix or interleaved into the attention loop ----
        open_ps = {}

        def emit_qk_unit(jb, kind, tb, half):
            wlist = wq_sb if kind == "q" else wk_sb
            key = (jb, kind, tb)
            if half == 0:
                open_ps[key] = ppsum.tile([P, NB], F32, tag="pp", name="pp")
            ps = open_ps[key]
            kh = KT // 2
            for k in range(kh * half, kh * half + kh):
                nc.tensor.matmul(
                    ps[:], wlist[k][:, jb * P:(jb + 1) * P],
                    hsT[k][:, tb * NB:(tb + 1) * NB],
                    start=(k == 0), stop=(k == KT - 1))
            if half == 1:
                del open_ps[key]
                if kind == "q":
                    bqc = 1 + JB + jb
                    nc.vector.scalar_tensor_tensor(
                        qpad[jb][0:HD, tb, 0, :], ps[0:HD, :],
                        cb_sb[0:HD, bqc:bqc + 1],
                        sexp[jb][0:HD, tb * NB:(tb + 1) * NB],
                        ALU.add, ALU.mult)
                    nc.vector.scalar_tensor_tensor(
                        qpad[jb][HD:P, tb, 1, :], ps[HD:P, :],
                        cb_sb[HD:P, bqc:bqc + 1],
                        sexp[jb][HD:P, tb * NB:(tb + 1) * NB],
                        ALU.add, ALU.mult)
                else:
                    nc.vector.tensor_scalar_add(
                        k_sb[jb][:, tb * NB:(tb + 1) * NB], ps[:], bk_sb[jb])

        def emit_v_unit(tc_, half):
            key = ("v", tc_)
            if half == 0:
                open_ps[key] = ppsum.tile([P, JL], F32, tag="pp", name="pp")
            ps = open_ps[key]
            kh = KT // 2
            for k in range(kh * half, kh * half + kh):
                nc.tensor.matmul(
                    ps[:], hsT[k][:, tc_ * P:(tc_ + 1) * P], wv_sb[k],
                    start=(k == 0), stop=(k == KT - 1))
            if half == 1:
                del open_ps[key]
                for h in range(NHL):
                    nc.vector.tensor_add(
                        v_sb[tc_][:, h, 0:HD], ps[:, h * HD:(h + 1) * HD],
                        bvb[:, h * HD:(h + 1) * HD])

        def emit_scale_unit(tb, half):
            """Compact scale chain for quarter tb -> tanh((x+bs)/2)."""
            key = ("sc", tb)
            if half == 0:
                open_ps[key] = ppsum.tile([P, NB], F32, tag="pp", name="pp")
            ps = open_ps[key]
            kh = KT // 2
            for k in range(kh * half, kh * half + kh):
                nc.tensor.matmul(
                    ps[0:GL, :], ws_sb[k],
                    hsT[k][:, tb * NB:(tb + 1) * NB],
                    start=(k == 0), stop=(k == KT - 1))
            if half == 1:
                del open_ps[key]
                nc.scalar.activation(sxc[0:GL, tb * NB:(tb + 1) * NB],
                                     ps[0:GL, :], AF.Tanh, bias=bs_sb,
                                     scale=0.5)

        def emit_sel_unit(jb, tb):
            """sexp[jb] quarter tb = SEL_jb^T @ sxc (x8 expand + affine)."""
            ps = ppsum.tile([P, NB], F32, tag="pp", name="pp")
            nc.tensor.matmul(
                ps[:], sel_sb[:, jb * P:(jb + 1) * P],
                sxc[:, tb * NB:(tb + 1) * NB], start=True, stop=True)
            nc.vector.tensor_copy(sexp[jb][:, tb * NB:(tb + 1) * NB], ps[:])

        # ---- prefix: quarter-0 work only, so window (0,0) starts early.
        # A garbage warmup burst (only needs wsall, the first DMA) ramps the
        # PE out of its cold p-state before the real chains arrive. ----
        warm = ppsum.tile([P, NB], F32, tag="pp", name="warm")
        wsmv = wsall[:].rearrange("p k c -> p (k c)")
        for r in range(8):
            nc.tensor.matmul(warm[0:GL, :], ws_sb[0], wsmv,
                             start=True, stop=True)
        for half in range(2):
            emit_scale_unit(0, half)
        # keep the PE clock hot while the K weights stream in (a stalled PE
        # drops to a slow p-state and the next ~3us of matmuls run 2x slow)
        warm2 = ppsum.tile([P, NB], F32, tag="pp", name="warm2")
        for r in range(28):
            nc.tensor.matmul(warm2[0:GL, :], ws_sb[0], wsmv,
                             start=True, stop=True)
        for half in range(2):
            emit_qk_unit(0, "k", 0, half)
        emit_sel_unit(0, 0)
        for half in range(2):
            emit_qk_unit(0, "q", 0, half)

        # deferred work, FIFO-drained as attention filler. Deadlines (window
        # (0,0), LAG=5): K-jb0-tb1 by iter 3, tb2 by iter 7, tb3 by iter 11;
        # V-unit c by iter c+4; scale/SEL/Q tb1 before window (0,1).
        projq = deque()
        for v in (0, 1):
            projq.append(("v", v, 0))
            projq.append(("v", v, 1))
        projq.append(("qk", 0, "k", 1, 0))
        projq.append(("qk", 0, "k", 1, 1))
        for v in (2, 3):
            projq.append(("v", v, 0))
            projq.append(("v", v, 1))
        projq.append(("qk", 0, "k", 2, 0))
        projq.append(("qk", 0, "k", 2, 1))
        for v in (4, 5):
            projq.append(("v", v, 0))
            projq.append(("v", v, 1))
        projq.append(("qk", 0, "k", 3, 0))
        projq.append(("qk", 0, "k", 3, 1))
        for v in range(6, 13):
            projq.append(("v", v, 0))
            projq.append(("v", v, 1))
        projq.append(("sc", 1, 0))
        projq.append(("sc", 1, 1))
        projq.append(("sel", 0, 1))
        projq.append(("qk", 0, "q", 1, 0))
        projq.append(("qk", 0, "q", 1, 1))
        for v in (13, 14, 15):
            projq.append(("v", v, 0))
            projq.append(("v", v, 1))
        # rest: per-quarter scale/SEL and remaining K/Q, ordered so each
        # piece lands >= one window before first use
        projq.append(("sc", 2, 0))
        projq.append(("sc", 2, 1))
        projq.append(("sel", 0, 2))
        projq.append(("qk", 0, "q", 2, 0))
        projq.append(("qk", 0, "q", 2, 1))
        projq.append(("sc", 3, 0))
        projq.append(("sc", 3, 1))
        projq.append(("sel", 0, 3))
        projq.append(("qk", 0, "q", 3, 0))
        projq.append(("qk", 0, "q", 3, 1))
        for tb in range(NTB):
            projq.append(("qk", 1, "k", tb, 0))
            projq.append(("qk", 1, "k", tb, 1))
        for tb in range(NTB):
            projq.append(("sel", 1, tb))
            projq.append(("qk", 1, "q", tb, 0))
            projq.append(("qk", 1, "q", tb, 1))
        for jb in (2, 3):
            for tb in range(NTB):
                projq.append(("qk", jb, "k", tb, 0))
                projq.append(("qk", jb, "k", tb, 1))
            for tb in range(NTB):
                projq.append(("sel", jb, tb))
                projq.append(("qk", jb, "q", tb, 0))
                projq.append(("qk", jb, "q", tb, 1))

        def drain_proj(n):
            for _ in range(n):
                if not projq:
                    return
                u = projq.popleft()
                if u[0] == "v":
                    emit_v_unit(u[1], u[2])
                elif u[0] == "sc":
                    emit_scale_unit(u[1], u[2])
                elif u[0] == "sel":
                    emit_sel_unit(u[1], u[2])
                else:
                    emit_qk_unit(u[1], u[2], u[3], u[4])

        # ================= attention =================
        # PSUM: scores ring 2x2 banks + ctxT 2x1 + pp 2x1 = 8 banks
        with tc.tile_pool(name="probs", bufs=2) as prpool, \
             tc.tile_pool(name="asm", bufs=2) as apool, \
             tc.tile_pool(name="spsum", bufs=2, space="PSUM") as spsum, \
             tc.tile_pool(name="cpsum", bufs=1, space="PSUM") as cpsum, \
             tc.tile_pool(name="rtmp", bufs=2) as rtpool:

            # per-(hp, ssb, iter) filler quota (pieces of ~4 matmuls):
            # window 0 takes the V/K bulk, windows 1-3 drain at 1/2 rate,
            # later (ACT-bound) windows at 1/3 rate to match PE slack
            def quota(hp, ssb, it):
                # deadline rule: every piece must drain >= one full window
                # before its first consumer is emitted
                w = hp * NSSB + ssb
                if w == 0:
                    return 2 if it < 8 else 3
                if w == 1 or w >= 4:
                    return 1 if it % 3 == 0 else 0
                return 1 if it % 2 == 0 else 0

            # deferred per-window tail work (normalize + output DMA),
            # drained one piece per iteration of the NEXT window. ctx is
            # accumulated TRANSPOSED ([s, d], probs chunk as stationary) so
            # no PE transpose / psum evacuation is needed; col 64 of each
            # head's slice is the softmax denominator.
            tailq = deque()

            def drain_tail(n):
                for _ in range(n):
                    if not tailq:
                        return
                    tailq.popleft()()

            def make_norm_tail(i, cps_i, asm):
                def run():
                    rc = rtpool.tile([P, W2 // P], F32, tag="rc")
                    nc.vector.reciprocal(rc[:], cps_i[:, :, HD])
                    for cc in range(W2 // P):
                        nc.vector.tensor_scalar_mul(
                            asm[:, cc, i, :], cps_i[:, cc, 0:HD],
                            rc[:, cc:cc + 1])
                return run

            def make_dma_tail(hp, ssb, cc, asm):
                def run():
                    srow = ssb * W2 + cc * P
                    nc.sync.dma_start(
                        out[srow:srow + P, hp * P:(hp + 1) * P], asm[:, cc])
                return run

            for hp in range(HP):
                for ssb in range(NSSB):
                    pts = prpool.tile([P, RING, 2 * W2], BF16, tag="pts",
                                      name="pts")
                    cps = [cpsum.tile([P, W2 // P, HD + 1], F32, tag=f"c{i}",
                                      name=f"c{i}") for i in range(2)]

                    def ctx_chunk(tcc):
                        # start=True clears the whole PSUM bank, so only the
                        # FIRST chain touching each head's bank sends it; the
                        # other cc chains' first writes overwrite anyway
                        # (has_written was cleared bank-wide).
                        for i in range(2):
                            h = hp * 2 + i
                            for cc in range(W2 // P):
                                nc.tensor.matmul(
                                    cps[i][:, cc, :],
                                    pts[:, tcc % RING,
                                        i * W2 + cc * P:i * W2 + (cc + 1) * P],
                                    v_sb[tcc][:, h, :],
                                    start=(tcc == 0 and cc == 0),
                                    stop=(tcc == NTC - 1),
                                    skip_group_check=True)

                    for tc_ in range(NTC):
                        if tc_ >= LAG:
                            ctx_chunk(tc_ - LAG)
                        drain_tail(1)
                        drain_proj(quota(hp, ssb, tc_))
                        sc = spsum.tile([P, 2 * W2], F32, tag="sc", name="sc")
                        for i in range(2):
                            nc.tensor.matmul(
                                sc[:, i * W2:(i + 1) * W2],
                                k_sb[hp][:, tc_ * P:(tc_ + 1) * P],
                                qpad[hp][:, ssb, i, :], start=True, stop=True)
                        nc.scalar.activation(
                            pts[:, tc_ % RING, :], sc[:], AF.Exp,
                            scale=1.0 / 8.0)
                    if (hp, ssb) == (0, 0):
                        drain_proj(3)
                    for tcc in range(NTC - LAG, NTC):
                        ctx_chunk(tcc)

                    asm = apool.tile([P, W2 // P, 2, HD], F32, tag="asm",
                                     name="asm")
                    for i in range(2):
                        tailq.append(make_norm_tail(i, cps[i], asm))
                    for cc in range(W2 // P):
                        tailq.append(make_dma_tail(hp, ssb, cc, asm))
            while tailq:
                tailq.popleft()()

    nc.finalize()
    return nc


_CACHE = {}


def _get_nc():
    if "nc" not in _CACHE:
        _CACHE["nc"] = build_bass()
    return _CACHE["nc"]


def _make_sel():
    """SEL [65, 512] for the tanh form: the device computes
    th = tanh((x + bs)/2) and sexp[jb][j, t] = a*th[c(jb,j), t] + m where
    a = (SC_MAX-SC_MIN)/2, m = (SC_MIN+SC_MAX)/2 (since
    sigmoid(x) = 0.5*tanh(x/2)+0.5), c(jb, j) = (2*jb + j//64)*8 + (j%64)//8.
    Row 64 multiplies the ones-row of sxc."""
    sel = np.zeros((65, 512), dtype=np.float32)
    for jb in range(4):
        for j in range(128):
            c = (2 * jb + j // 64) * 8 + (j % 64) // 8
            sel[c, jb * 128 + j] = (SC_MAX - SC_MIN) / 2
    sel[64, :] = (SC_MIN + SC_MAX) / 2
    return sel


def _shard(inputs):
    """Split full inputs into 8 per-core input maps (host-side, bf16 cast)."""
    hidden_states = inputs["hidden_states"]
    Wq, bq = inputs["Wq"], inputs["bq"]
    Wk, bk = inputs["Wk"], inputs["bk"]
    Wv, bv = inputs["Wv"], inputs["bv"]
    Ws, bs = inputs["Ws"], inputs["bs"]
    JL = 512   # output cols per core
    GL = 64    # Ws cols per core
    bf = ml_dtypes.bfloat16
    sel = _make_sel().astype(bf)
    in_maps = []
    for c in range(8):
        b, hh = c // 2, c % 2
        # ws packed to [128, 8*64] (one contiguous DMA on device)
        wsl = Ws[:, hh * GL:(hh + 1) * GL].reshape(8, 128, GL)
        wsl = np.ascontiguousarray(wsl.transpose(1, 0, 2)).reshape(128, -1)
        # cb packs bs/2, bk, bq as per-partition columns [128, 9]
        cbm = np.zeros((128, 9), dtype=np.float32)
        cbm[0:GL, 0] = 0.5 * bs[hh * GL:(hh + 1) * GL]
        for jb in range(4):
            cbm[:, 1 + jb] = bk[hh * JL + jb * 128:hh * JL + (jb + 1) * 128]
            cbm[:, 5 + jb] = bq[hh * JL + jb * 128:hh * JL + (jb + 1) * 128]
        in_maps.append({
            "hs": hidden_states[b].T.astype(bf),
            "wq": np.ascontiguousarray(Wq[:, hh * JL:(hh + 1) * JL]).astype(bf),
            "wk": np.ascontiguousarray(Wk[:, hh * JL:(hh + 1) * JL]).astype(bf),
            "wv": np.ascontiguousarray(Wv[:, hh * JL:(hh + 1) * JL]).astype(bf),
            "ws": wsl.astype(bf),
            "cb": cbm,
            "bs": (0.5 * bs[hh * GL:(hh + 1) * GL]).astype(np.float32),
            "bv": np.ascontiguousarray(bv[hh * JL:(hh + 1) * JL]).astype(np.float32),
            "sel": sel,
        })
    return in_maps


def kernel(hidden_states, Wq, bq, Wk, bk, Wv, bv, Ws, bs):
    B, S, H = hidden_states.shape
    NH = 16
    JL = 512
    nc = _get_nc()
    in_maps = _shard(dict(hidden_states=hidden_states, Wq=Wq, bq=bq, Wk=Wk,
                          bk=bk, Wv=Wv, bv=bv, Ws=Ws, bs=bs))

    res = bass_utils.run_bass_kernel_spmd(nc, in_maps, core_ids=list(range(8)))

    outp = np.zeros((B, S, NH * HD), dtype=np.float32)
    for c in range(8):
        b, hh = c // 2, c % 2
        outp[b][:, hh * JL:(hh + 1) * JL] = res.results[c]["out"]
    return outp

